# revision 17
# baseline (speedup 1.0000x reference)
"""Trainium2 Bass kernel for nn_CausalAttention (B=4, T=2048, d_model=1024, d_ff=2048).

Sharding: 8 cores = 4 batches x 2 query-halves. Each core owns 8 query blocks
of 128 rows, paired so causal work is balanced and the per-core program is
IDENTICAL (SPMD): the k-th owned block always computes E[k] key chunks; exact
causal masking arrives as per-core input data.

Math identities (v3):
  S   = q@k.T = x (Wq Wk.T) x.T   (contract d_model via M = Wq Wk.T)
  out = softmax(S) @ x @ (Wv Wf) + bf = (P @ x) @ N2 + bf
The v-path is re-associated: o1 = P@x (contracts keys, 512-col streams with
the transposed-P stationary reused), o1 is PE-transposed, and out = o1T@N2.
This removes the whole vf = x@N2 phase and its pair-AllGathers; N2 is needed
only at the very end, hiding its AllGather completely.

Input-independent weight products are sharded and AllGathered (on-chip):
  M  = Wq @ Wk.T  - each core computes a 128-row slice (fp16, row-shard)
  N2 = Wv @ Wf    - each core computes a 256-col slice (bf16, col-shard)

DMA: every operand is host-marshalled into its exact SBUF layout
[128 part, chunk, col] and loaded with ONE large dma_start (a single DMA
splits across all 16 SDMA engines: 1MB+ transfers run at ~350-420GB/s,
while 256KB chunked loads pay ~2.6us each ~= 100GB/s). Loads are spread
over the sync/scalar/vector/gpsimd queues by deadline.

Schedule is engineered for an uninterrupted PE stream (TRN2's PE drops to
~1.2GHz for ~3us after every idle gap, so idle hurts twice):
  - memset-backed dummy matmuls at t=0 ramp the PE clock while loads stream
  - PE order: M -> N2 -> uT -> attention; the M AllGather + mM load-back
    completes behind the N2 matmuls so uT starts PE-bound
  - attention stages interleave (tr_k, o1_k, out_k, sm_{k+2}) so exp
    latency is hidden; PSUM fits 8 banks exactly
    (sps 2x[128,512] | o1/out shared 2x[128,1024] | pt 2x[128,128]).

The score pipeline (M, uT = xq M, S = uT.T x.T) runs in fp16 (PE fp16 is
1 cycle/row; fp16's 11-bit mantissa matches the fp32r HW rounding anyway).
Probabilities and the o1/out path are bf16 with fp32 PSUM accumulation.
Output is stored bf16 (within the 2e-2 budget) and cast to fp32 on host.
"""

import sys
from contextlib import ExitStack

for _p in ("/opt/trn_rl_repo", "/root/.axon_site/_ro/trn_rl_repo"):
    if _p not in sys.path:
        sys.path.append(_p)

import ml_dtypes
import numpy as np

import concourse.bass as bass
import concourse.mybir as mybir
import concourse.tile as tile
from concourse import bacc
from concourse.bass_utils import run_bass_kernel_spmd
from concourse.masks import make_identity

F32 = mybir.dt.float32
F16 = mybir.dt.float16
BF16 = mybir.dt.bfloat16

B, T, C, F = 4, 2048, 1024, 2048
NB = T // 128  # 16 query/key blocks per batch
CC = C // 128  # 8 chunks of d_model
FC = F // 128  # 16 chunks of d_ff
NCORES = 8

# Stage k owns DEVICE block 15-2k (always odd): half 0 sees x unpermuted
# (owns true odd blocks), half 1 sees x with adjacent 128-row blocks pair-
# swapped (owns true even blocks). This keeps the program SPMD-identical
# AND makes the owned query columns of xT a uniform stride-256 pattern, so
# uT reads them straight out of xT (no separate xqT tensor or load).
# E[k] = 16-2k key chunks computed for stage k; sum(E)=72 (ideal 68).
E = [16, 14, 12, 10, 8, 6, 4, 2]
NEG = -1.0e30

ALL8 = [list(range(8))]

_CACHE = {}


def _build_program():
    """Trace + finalize the (single, SPMD) Bass program."""
    nc = bacc.Bacc(None)

    # all operands arrive in SBUF layout [128, chunk, col] from the host
    xT_ext = nc.declare_dram_parameter("xTin", [128, CC, T], F16, isOutput=False)
    m2_ext = nc.declare_dram_parameter("mask2", [128, 8, 256], BF16, isOutput=False)
    wqs_ext = nc.declare_dram_parameter("WqTs", [128, FC, 128], F16, isOutput=False)
    wkT_ext = nc.declare_dram_parameter("WkTf", [128, FC, C], F16, isOutput=False)
    wvT_ext = nc.declare_dram_parameter("WvTb", [128, FC, C], BF16, isOutput=False)
    wfs_ext = nc.declare_dram_parameter("Wfs", [128, FC, 256], BF16, isOutput=False)
    bf_ext = nc.declare_dram_parameter("bf", [F], F32, isOutput=False)
    out_ext = nc.declare_dram_parameter("out", [8, 128, F], BF16, isOutput=True)

    with tile.TileContext(nc) as tc, ExitStack() as root:
        persist = root.enter_context(tc.tile_pool(name="persist", bufs=1))
        dram = root.enter_context(tc.tile_pool(name="dram", bufs=1, space="DRAM"))

        identf = persist.tile([128, 128], F16, tag="identf")
        make_identity(nc, identf[:, :])

        # long-lived operands (loads emitted late, where first needed)
        xT = persist.tile([128, CC, T], F16, tag="xT")  # 32KB/part
        xb = persist.tile([128, NB, C], F16, tag="xb")  # 32KB/part
        uT = persist.tile([128, CC, 1024], F16, tag="uT")  # 16KB/part
        bfb = persist.tile([128, F], F32, tag="bfb")  # 8KB/part
        m2 = persist.tile([128, 8, 256], BF16, tag="m2")  # 4KB/part
        EOFF = [sum(E[:k]) for k in range(8)]
        ptall = persist.tile([128, sum(E), 128], F16, tag="ptall")  # 18KB

        # exp table warm (a dummy exp shaped exactly like the real softmax
        # exp); PE warmup uses identbf (ready at t~0, no load dependency).
        wrm_b = persist.tile([128, 1], F32, tag="wrm_b")
        wrm_acc = persist.tile([128, 1], F32, tag="wrm_acc")
        nc.vector.tensor_copy(out=wrm_b, in_=identf[:, :1])
        nc.scalar.activation(
            out=xb[:, 0:2, :],
            in_=bfb[:, :],
            func=mybir.ActivationFunctionType.Exp,
            bias=wrm_b,
            scale=-200.0,
            accum_out=wrm_acc,
        )
        # (xqT load is emitted after phase 1 on the scalar queue; xb load is
        # emitted inside phase 2 on the gpsimd queue.)

        # collective buffers (DRAM)
        msl_d = dram.tile([128, C], F16, tag="msl_d")
        mall_d = dram.tile([C, C], F16, tag="mall_d", addr_space="Shared")
        n2s_d = dram.tile([128, F], F16, tag="n2s_d")
        n2all_d = dram.tile([NCORES * 128, F], F16, tag="n2all_d", addr_space="Shared")

        # ======== phase 1: M-rows = Wq[my 128 rows] @ WkT (fp16), AllGather
        with ExitStack() as ph1, nc.named_scope("p1_M"):
            wqp = ph1.enter_context(tc.tile_pool(name="wqp", bufs=1))
            ps1 = ph1.enter_context(tc.tile_pool(name="ps1", bufs=1, space="PSUM"))
            wqs = wqp.tile([128, FC, 128], F16, tag="wqs")  # 4KB/part
            wkT = wqp.tile([128, FC, C], F16, tag="wkT")  # 32KB/part
            # wkT in four 1MB pieces across the two fast queues so the
            # first chunks land ~16us; M's f-order follows arrival order.
            nc.scalar.dma_start(out=wqs[:, :, :], in_=wqs_ext[:, :, :])
            nc.sync.dma_start(out=wkT[:, 0:4, :], in_=wkT_ext[:, 0:4, :])
            nc.scalar.dma_start(out=wkT[:, 8:12, :], in_=wkT_ext[:, 8:12, :])
            nc.sync.dma_start(out=wkT[:, 4:8, :], in_=wkT_ext[:, 4:8, :])
            nc.scalar.dma_start(out=wkT[:, 12:16, :], in_=wkT_ext[:, 12:16, :])

            # PE p-state warmup: dummy matmuls with no load dependencies
            wrmp = ph1.enter_context(tc.tile_pool(name="wrmp", bufs=1, space="PSUM"))
            wps = wrmp.tile([128, 128], F32, tag="wps")
            for _ in range(40):
                nc.tensor.matmul(wps[:, :], identf[:, :], identf[:, :])

            msl_sb = wqp.tile([128, C], F16, tag="msl_sb")  # 2KB/part
            mps = ps1.tile([128, C], F32, tag="mps")  # 2 banks
            forder = [0, 1, 2, 3, 8, 9, 10, 11, 4, 5, 6, 7, 12, 13, 14, 15]
            for i, f in enumerate(forder):
                for h in range(2):
                    nc.tensor.matmul(
                        mps[:, h * 512 : (h + 1) * 512],
                        wqs[:, f, :],
                        wkT[:, f, h * 512 : (h + 1) * 512],
                        start=(i == 0),
                        stop=(i == FC - 1),
                    )
            nc.vector.tensor_copy(out=msl_sb[:, :], in_=mps[:, :])
            nc.gpsimd.dma_start(out=msl_d[:, :], in_=msl_sb[:, :])
            # (the M AllGather itself is emitted in phase 2, on the gpsimd
            # queue after the wvT/wfs loads, so its wait doesn't stall them)

        # wvT halves ride the two fast HWDGE queues (SWDGE is ~64GB/s)

        # ======== phase 2: N2-slice = Wv @ Wf[:, my 256 cols], AllGather ===
        with ExitStack() as ph2, nc.named_scope("p2_N2"):
            wvp = ph2.enter_context(tc.tile_pool(name="wvp", bufs=1))
            ps2 = ph2.enter_context(tc.tile_pool(name="ps2", bufs=1, space="PSUM"))
            wvT = wvp.tile([128, FC, C], BF16, tag="wvT")  # 32KB/part
            wfs = wvp.tile([128, FC, 256], BF16, tag="wfs")  # 8KB/part
            nc.gpsimd.dma_start(out=wfs[:, :, :], in_=wfs_ext[:, :, :])
            nc.scalar.dma_start(out=wvT[:, 8:, :], in_=wvT_ext[:, 8:, :])
            nc.sync.dma_start(out=wvT[:, :8, :], in_=wvT_ext[:, :8, :])
            # sync: xT behind wvT-A (scores need it ~t=80)
            nc.sync.dma_start(out=xT[:, :, :], in_=xT_ext[:, :, :])
            # M AllGather on the gpsimd queue (its wait blocks nothing here)
            nc.gpsimd.collective_compute(
                "AllGather",
                mybir.AluOpType.bypass,
                replica_groups=ALL8,
                ins=[msl_d[:, :]],
                outs=[mall_d[:, :]],
            )
            n2s_sb = wvp.tile([128, CC, 256], F16, tag="n2s_sb")  # 4KB/part
            for ah in range(2):
                nps = ps2.tile([128, 4, 512], F32, tag="nps", name=f"nps{ah}")
                for f in range(FC):
                    for a4 in range(4):
                        nc.tensor.matmul(
                            nps[:, a4, :256],
                            wvT[:, f, (ah * 4 + a4) * 128 : (ah * 4 + a4 + 1) * 128],
                            wfs[:, f, :],
                            start=(f == 0),
                            stop=(f == FC - 1),
                        )
                for a4 in range(4):
                    nc.vector.tensor_copy(
                        out=n2s_sb[:, ah * 4 + a4, :], in_=nps[:, a4, :256]
                    )
            nc.scalar.dma_start(out=n2s_d[:, :], in_=n2s_sb[:, :, :])
            nc.gpsimd.collective_compute(
                "AllGather",
                mybir.AluOpType.bypass,
                replica_groups=ALL8,
                ins=[n2s_d[:, :]],
                outs=[n2all_d[:, :]],
            )

        # ======== phase 2b: xb[t-part, c] = PE-transpose of xT ==============
        # (replaces a 4MB HBM load; fills the PE while the M-AllGather runs)
        with ExitStack() as ph2b, nc.named_scope("p2b_xb"):
            psx = ph2b.enter_context(tc.tile_pool(name="psx", bufs=2, space="PSUM"))
            for s in range(NB):
                for cc in range(CC):
                    xtp = psx.tile(
                        [128, 128], F16, tag="xtp", name=f"xtp{s}_{cc}"
                    )
                    nc.tensor.transpose(
                        xtp[:, :], xT[:, cc, s * 128 : (s + 1) * 128], identf[:, :]
                    )
                    nc.vector.tensor_copy(
                        out=xb[:, s, cc * 128 : (cc + 1) * 128], in_=xtp[:, :]
                    )

        # ======== phase 3: uT = (xq M).T  [c2-chunk, owned-t] fp16 =========
        # post pool reuses the weight pools' SBUF space (they are closed).
        post = root.enter_context(tc.tile_pool(name="post", bufs=1))
        mM = post.tile([128, CC, C], F16, tag="mM")  # 16KB/part
        n2 = post.tile([128, NCORES, F], F16, tag="n2")  # 32KB/part
        with ExitStack() as ph3, nc.named_scope("p3_uT"):
            ps3 = ph3.enter_context(tc.tile_pool(name="ps3", bufs=4, space="PSUM"))
            # mall_d row r = M row r -> mM[p, c1, :] = M[c1*128+p, :]
            for mh in range(2):  # c2 halves: uT c2 0-3 start on the first
                nc.scalar.dma_start(
                    out=mM[:, :, mh * 512 : (mh + 1) * 512],
                    in_=mall_d[:, mh * 512 : (mh + 1) * 512].rearrange(
                        "(c p) w -> p c w", p=128
                    ),
                )
            def xq_ap(c1, tt):
                """owned (odd) device blocks {1,3,5,7}+8*tt of xT[:, c1, :]"""
                base = xT[:, c1, :]
                return bass.AP(
                    tensor=base.tensor,
                    offset=base.offset + 128 + tt * 1024,
                    ap=[base.ap[0], [256, 4], [1, 128]],
                )

            for tt in (1, 0):  # tt=1 first: it feeds stages 0-3
                for c2 in range(CC):
                    ups = ps3.tile(
                        [128, 512], F32, tag="ups", name=f"ups{tt}_{c2}"
                    )
                    for c1 in range(CC):
                        nc.tensor.matmul(
                            ups[:, :],
                            mM[:, c1, c2 * 128 : (c2 + 1) * 128],
                            xq_ap(c1, tt),
                            start=(c1 == 0),
                            stop=(c1 == CC - 1),
                        )
                    nc.vector.tensor_copy(
                        out=uT[:, c2, tt * 512 : (tt + 1) * 512], in_=ups[:, :]
                    )

        # small loads (act queue is idle now); n2 gather cache on sync.
        # n2[p, gi, cj*256+w] = N2[cj*128+p, gi*256+w] -- row-block gi of
        # n2all_d lands contiguously (4KB runs per partition).
        nc.scalar.dma_start(out=m2[:, :, :], in_=m2_ext[:, :, :])
        bf_ap = bf_ext[:]
        nc.scalar.dma_start(
            out=bfb,
            in_=bass.AP(
                tensor=bf_ap.tensor,
                offset=bf_ap.offset,
                ap=[[0, 128]] + list(bf_ap.ap),
            ),
        )
        nc.sync.dma_start(
            out=n2[:, :, :],
            in_=n2all_d[:, :].rearrange("(g p) w -> p g w", p=128),
        )

        # ======== phase 5: attention over owned blocks =====================
        with ExitStack() as ph6:
            st6 = ph6.enter_context(tc.tile_pool(name="st6", bufs=2))
            small = ph6.enter_context(tc.tile_pool(name="small", bufs=4))
            ps_s = ph6.enter_context(tc.tile_pool(name="ps_s", bufs=2, space="PSUM"))
            ps_big = ph6.enter_context(tc.tile_pool(name="ps_big", bufs=2, space="PSUM"))
            ps_t = ph6.enter_context(tc.tile_pool(name="ps_t", bufs=2, space="PSUM"))

            def softmax_stage(k):
                with nc.named_scope(f"p5_sm{k}"):
                    return _softmax_stage(k)

            def _softmax_stage(k):
                """scores -> masked SBUF copy -> exp -> bf16 probs"""
                ek = E[k]
                scols = ek * 128
                s_sb = st6.tile([128, T], F32, tag="s_sb", name=f"s_sb{k}", bufs=2)
                m0 = scols - 256  # mask window start
                for h0 in range(0, scols, 512):
                    hw = min(512, scols - h0)
                    sps = ps_s.tile([128, 512], F32, tag="sps", name=f"sps{k}_{h0}")
                    for c2 in range(CC):
                        nc.tensor.matmul(
                            sps[:, :hw],
                            uT[:, c2, (7 - k) * 128 : (8 - k) * 128],
                            xT[:, c2, h0 : h0 + hw],
                            start=(c2 == 0),
                            stop=(c2 == CC - 1),
                        )
                    plain = min(hw, max(0, m0 - h0))
                    if plain > 0:
                        nc.vector.tensor_copy(
                            out=s_sb[:, h0 : h0 + plain], in_=sps[:, :plain]
                        )
                    if plain < hw:
                        nc.vector.tensor_add(
                            s_sb[:, h0 + plain : h0 + hw],
                            sps[:, plain:hw],
                            m2[:, k, h0 + plain - m0 : h0 + hw - m0],
                        )
                negmax = small.tile(
                    [128, 1], F32, tag="negmax", name=f"negmax{k}", bufs=6
                )
                nc.vector.tensor_reduce(
                    out=negmax,
                    in_=s_sb[:, :scols],
                    axis=mybir.AxisListType.X,
                    op=mybir.AluOpType.max,
                    negate=True,
                )
                psb = st6.tile([128, T], F16, tag="psb", name=f"psb{k}", bufs=2)
                rsum = small.tile([128, 1], F32, tag="rsum", name=f"rsum{k}", bufs=6)
                nc.scalar.activation(
                    out=psb[:, :scols],
                    in_=s_sb[:, :scols],
                    func=mybir.ActivationFunctionType.Exp,
                    bias=negmax,
                    scale=1.0,
                    accum_out=rsum,
                )
                rinv = small.tile([128, 1], F32, tag="rinv", name=f"rinv{k}", bufs=6)
                nc.vector.reciprocal(out=rinv, in_=rsum)
                return psb, rinv

            def transpose_stage(k, psb):
                with nc.named_scope(f"p5_tr{k}"):
                    for sc in range(E[k]):
                        pt = ps_t.tile([128, 128], F16, tag="pt", name=f"pt{k}_{sc}")
                        nc.tensor.transpose(
                            pt[:, :], psb[:, sc * 128 : (sc + 1) * 128], identf[:, :]
                        )
                        if k < 4:  # Act handles long stages; DVE the short
                            nc.scalar.activation(  # ones (Act is busy w/ exp)
                                out=ptall[:, EOFF[k] + sc, :],
                                in_=pt[:, :],
                                func=mybir.ActivationFunctionType.Copy,
                            )
                        else:
                            nc.vector.tensor_copy(
                                out=ptall[:, EOFF[k] + sc, :], in_=pt[:, :]
                            )

            def o1_stage(k):
                """o1 = P @ x [q,1024] -> bf16 -> PE transpose -> o1T [c,q]"""
                with nc.named_scope(f"p5_o1_{k}"):
                    ek = E[k]
                    o1ps = ps_big.tile([128, 1024], F32, tag="big", name=f"o1ps{k}")
                    for sc in range(ek):
                        for h in range(2):  # same ptall stationary twice
                            nc.tensor.matmul(
                                o1ps[:, h * 512 : (h + 1) * 512],
                                ptall[:, EOFF[k] + sc, :],
                                xb[:, sc, h * 512 : (h + 1) * 512],
                                start=(sc == 0),
                                stop=(sc == ek - 1),
                            )
                    o1sb = st6.tile([128, C], F16, tag="o1sb", name=f"o1sb{k}", bufs=1)
                    nc.scalar.activation(
                        out=o1sb[:, :],
                        in_=o1ps[:, :],
                        func=mybir.ActivationFunctionType.Copy,
                    )
                    o1T = st6.tile(
                        [128, CC, 128], F16, tag="o1T", name=f"o1T{k}", bufs=1
                    )
                    for cj in range(CC):
                        o1tp = ps_t.tile(
                            [128, 128], F16, tag="pt", name=f"o1tp{k}_{cj}"
                        )
                        nc.tensor.transpose(
                            o1tp[:, :], o1sb[:, cj * 128 : (cj + 1) * 128], identf[:, :]
                        )
                        nc.scalar.activation(
                            out=o1T[:, cj, :],
                            in_=o1tp[:, :],
                            func=mybir.ActivationFunctionType.Copy,
                        )
                    return o1T

            def out_stage(k, o1T, rinv):
                """out = (o1T @ N2) * rinv + bf, store bf16"""
                with nc.named_scope(f"p5_out{k}"):
                    orow = st6.tile([128, F], BF16, tag="orow", name=f"orow{k}", bufs=1)
                    for h2 in range(2):
                        ops = ps_big.tile(
                            [128, 1024], F32, tag="big", name=f"ops{k}_{h2}"
                        )
                        for cj in range(CC):
                            for g2 in range(2):  # same o1T stationary twice
                                gi = (h2 * 2 + g2) * 2
                                nc.tensor.matmul(
                                    ops[:, g2 * 512 : (g2 + 1) * 512],
                                    o1T[:, cj, :],
                                    n2[:, gi : gi + 2, cj * 256 : (cj + 1) * 256],
                                    start=(cj == 0),
                                    stop=(cj == CC - 1),
                                )
                        nc.vector.scalar_tensor_tensor(
                            out=orow[:, h2 * 1024 : (h2 + 1) * 1024],
                            in0=ops,
                            scalar=rinv,
                            in1=bfb[:, h2 * 1024 : (h2 + 1) * 1024],
                            op0=mybir.AluOpType.mult,
                            op1=mybir.AluOpType.add,
                        )
                    nc.sync.dma_start(out=out_ext[k], in_=orow)

            # pipeline: S_{k+2} runs on PE between stage-k transposes and
            # stage-(k+1) work so exp latency never stalls the PE.
            sm = {}
            sm[0] = softmax_stage(0)
            sm[1] = softmax_stage(1)
            for k in range(8):
                psb, rinv = sm.pop(k)
                transpose_stage(k, psb)
                o1T = o1_stage(k)
                out_stage(k, o1T, rinv)
                if k + 2 < 8:
                    sm[k + 2] = softmax_stage(k + 2)

    nc.finalize()
    return nc


def _get_program():
    if "nc" not in _CACHE:
        _CACHE["nc"] = _build_program()
    return _CACHE["nc"]


def _sb(a, p=128):
    """[n*p, w] -> SBUF layout [p, n, w] (partition-major)."""
    n = a.shape[0] // p
    return np.ascontiguousarray(a.reshape(n, p, a.shape[1]).transpose(1, 0, 2))


def _make_in_maps(x, Wq, Wk, Wv, Wf, bf):
    x = np.ascontiguousarray(x, dtype=np.float32)
    WqT = np.ascontiguousarray(np.asarray(Wq, dtype=np.float32).T)
    WkT16 = np.ascontiguousarray(np.asarray(Wk, dtype=np.float32).T).astype(np.float16)
    WvTb = np.ascontiguousarray(np.asarray(Wv, dtype=np.float32).T).astype(
        ml_dtypes.bfloat16
    )
    Wfb = np.asarray(Wf, dtype=np.float32).astype(ml_dtypes.bfloat16)
    bf = np.ascontiguousarray(bf, dtype=np.float32)
    wkT_sb = _sb(WkT16)
    wvT_sb = _sb(WvTb)
    in_maps = []
    for core in range(NCORES):
        b, h = core // 2, core % 2
        xb = x[b]
        if h == 1:  # pair-swap adjacent 128-row blocks: device block 2j+1
            xb = np.ascontiguousarray(  # holds true even block 2j
                xb.reshape(8, 2, 128, C)[:, ::-1].reshape(T, C)
            )
        mask2 = np.zeros((8, 128, 256), dtype=np.float32)  # cast below
        for k in range(8):
            tq = 15 - 2 * k - h  # true query block of stage k
            sd = (E[k] - 2) * 128 + np.arange(256)[None, :]  # device key col
            s = (np.right_shift(sd, 7) ^ h) * 128 + (sd & 127)  # true key
            t = tq * 128 + np.arange(128)[:, None]
            mask2[k] = np.where(s <= t, 0.0, NEG).astype(np.float32)
        in_maps.append(
            {
                "xTin": _sb(np.ascontiguousarray(xb.T).astype(np.float16)),
                "mask2": np.ascontiguousarray(
                    mask2.astype(ml_dtypes.bfloat16).transpose(1, 0, 2)
                ),
                "WqTs": _sb(
                    np.ascontiguousarray(
                        WqT[:, core * 128 : (core + 1) * 128]
                    ).astype(np.float16)
                ),
                "WkTf": wkT_sb,
                "WvTb": wvT_sb,
                "Wfs": _sb(
                    np.ascontiguousarray(Wfb[:, core * 256 : (core + 1) * 256])
                ),
                "bf": bf,
            }
        )
    return in_maps


def run_on_hw(inputs, trace=False, trace_cores=None):
    nc = _get_program()
    in_maps = _make_in_maps(**inputs)
    res = run_bass_kernel_spmd(
        nc, in_maps, list(range(NCORES)), trace=trace, trace_cores=trace_cores
    )
    out = np.empty((B, T, F), dtype=np.float32)
    for core in range(NCORES):
        b, h = core // 2, core % 2
        o = res.results[core]["out"]  # [8, 128, F] bf16
        for k in range(8):
            blk = 15 - 2 * k - h  # true query block of stage k
            out[b, blk * 128 : (blk + 1) * 128, :] = o[k].astype(np.float32)
    return out, res


def kernel(x, Wq, Wk, Wv, Wf, bf):
    out, _ = run_on_hw(dict(x=x, Wq=Wq, Wk=Wk, Wv=Wv, Wf=Wf, bf=bf))
    return out


# revision 18
# speedup vs baseline: 1.0272x; 1.0272x over previous
"""Trainium2 Bass kernel for nn_CausalAttention (B=4, T=2048, d_model=1024, d_ff=2048).

Sharding: 8 cores = 4 batches x 2 query-halves. Each core owns 8 query blocks
of 128 rows, paired so causal work is balanced and the per-core program is
IDENTICAL (SPMD): the k-th owned block always computes E[k] key chunks; exact
causal masking arrives as per-core input data.

Math identities (v3):
  S   = q@k.T = x (Wq Wk.T) x.T   (contract d_model via M = Wq Wk.T)
  out = softmax(S) @ x @ (Wv Wf) + bf = (P @ x) @ N2 + bf
The v-path is re-associated: o1 = P@x (contracts keys, 512-col streams with
the transposed-P stationary reused), o1 is PE-transposed, and out = o1T@N2.
This removes the whole vf = x@N2 phase and its pair-AllGathers; N2 is needed
only at the very end, hiding its AllGather completely.

Input-independent weight products are sharded and AllGathered (on-chip):
  M  = Wq @ Wk.T  - each core computes a 128-row slice (fp16, row-shard)
  N2 = Wv @ Wf    - each core computes a 256-col slice (bf16, col-shard)

DMA: every operand is host-marshalled into its exact SBUF layout
[128 part, chunk, col] and loaded with ONE large dma_start (a single DMA
splits across all 16 SDMA engines: 1MB+ transfers run at ~350-420GB/s,
while 256KB chunked loads pay ~2.6us each ~= 100GB/s). Loads are spread
over the sync/scalar/vector/gpsimd queues by deadline.

Schedule is engineered for an uninterrupted PE stream (TRN2's PE drops to
~1.2GHz for ~3us after every idle gap, so idle hurts twice):
  - memset-backed dummy matmuls at t=0 ramp the PE clock while loads stream
  - PE order: M -> N2 -> uT -> attention; the M AllGather + mM load-back
    completes behind the N2 matmuls so uT starts PE-bound
  - attention stages interleave (tr_k, o1_k, out_k, sm_{k+2}) so exp
    latency is hidden; PSUM fits 8 banks exactly
    (sps 2x[128,512] | o1/out shared 2x[128,1024] | pt 2x[128,128]).

The score pipeline (M, uT = xq M, S = uT.T x.T) runs in fp16 (PE fp16 is
1 cycle/row; fp16's 11-bit mantissa matches the fp32r HW rounding anyway).
Probabilities and the o1/out path are bf16 with fp32 PSUM accumulation.
Output is stored bf16 (within the 2e-2 budget) and cast to fp32 on host.
"""

import sys
from contextlib import ExitStack

for _p in ("/opt/trn_rl_repo", "/root/.axon_site/_ro/trn_rl_repo"):
    if _p not in sys.path:
        sys.path.append(_p)

import ml_dtypes
import numpy as np

import concourse.bass as bass
import concourse.mybir as mybir
import concourse.tile as tile
from concourse import bacc
from concourse.bass_utils import run_bass_kernel_spmd
from concourse.masks import make_identity

F32 = mybir.dt.float32
F16 = mybir.dt.float16
BF16 = mybir.dt.bfloat16

B, T, C, F = 4, 2048, 1024, 2048
NB = T // 128  # 16 query/key blocks per batch
CC = C // 128  # 8 chunks of d_model
FC = F // 128  # 16 chunks of d_ff
NCORES = 8

# Stage k owns DEVICE block 15-2k (always odd): half 0 sees x unpermuted
# (owns true odd blocks), half 1 sees x with adjacent 128-row blocks pair-
# swapped (owns true even blocks). This keeps the program SPMD-identical
# AND makes the owned query columns of xT a uniform stride-256 pattern, so
# uT reads them straight out of xT (no separate xqT tensor or load).
# E[k] = 16-2k key chunks computed for stage k; sum(E)=72 (ideal 68).
E = [16, 14, 12, 10, 8, 6, 4, 2]
NEG = -1.0e30

ALL8 = [list(range(8))]

_CACHE = {}


def _build_program():
    """Trace + finalize the (single, SPMD) Bass program."""
    nc = bacc.Bacc(None)

    # all operands arrive in SBUF layout [128, chunk, col] from the host
    xT_ext = nc.declare_dram_parameter("xTin", [128, CC, T], F16, isOutput=False)
    m2_ext = nc.declare_dram_parameter("mask2", [128, 8, 256], BF16, isOutput=False)
    wqs_ext = nc.declare_dram_parameter("WqTs", [128, FC, 128], F16, isOutput=False)
    wkT_ext = nc.declare_dram_parameter("WkTf", [128, FC, C], F16, isOutput=False)
    wvT_ext = nc.declare_dram_parameter("WvTb", [128, FC, C], BF16, isOutput=False)
    wfs_ext = nc.declare_dram_parameter("Wfs", [128, FC, 256], BF16, isOutput=False)
    bf_ext = nc.declare_dram_parameter("bf", [F], F32, isOutput=False)
    out_ext = nc.declare_dram_parameter("out", [8, 128, F], BF16, isOutput=True)

    with tile.TileContext(nc) as tc, ExitStack() as root:
        persist = root.enter_context(tc.tile_pool(name="persist", bufs=1))
        dram = root.enter_context(tc.tile_pool(name="dram", bufs=1, space="DRAM"))

        identf = persist.tile([128, 128], F16, tag="identf")
        make_identity(nc, identf[:, :])

        # long-lived operands (loads emitted late, where first needed)
        xT = persist.tile([128, CC, T], F16, tag="xT")  # 32KB/part
        xb = persist.tile([128, NB, C], F16, tag="xb")  # 32KB/part
        uT = persist.tile([128, CC, 1024], F16, tag="uT")  # 16KB/part
        bfb = persist.tile([128, F], F32, tag="bfb")  # 8KB/part
        m2 = persist.tile([128, 8, 256], BF16, tag="m2")  # 4KB/part
        EOFF = [sum(E[:k]) for k in range(8)]
        ptall = persist.tile([128, sum(E), 128], F16, tag="ptall")  # 18KB

        # exp table warm (a dummy exp shaped exactly like the real softmax
        # exp); PE warmup uses identbf (ready at t~0, no load dependency).
        wrm_b = persist.tile([128, 1], F32, tag="wrm_b")
        wrm_acc = persist.tile([128, 1], F32, tag="wrm_acc")
        nc.vector.tensor_copy(out=wrm_b, in_=identf[:, :1])
        nc.scalar.activation(
            out=xb[:, 0:2, :],
            in_=bfb[:, :],
            func=mybir.ActivationFunctionType.Exp,
            bias=wrm_b,
            scale=-200.0,
            accum_out=wrm_acc,
        )
        # (xqT load is emitted after phase 1 on the scalar queue; xb load is
        # emitted inside phase 2 on the gpsimd queue.)

        # collective buffers (DRAM)
        msl_d = dram.tile([128, C], F16, tag="msl_d")
        mall_d = dram.tile([C, C], F16, tag="mall_d", addr_space="Shared")
        n2s_d = dram.tile([128, F], F16, tag="n2s_d")
        n2all_d = dram.tile([NCORES * 128, F], F16, tag="n2all_d", addr_space="Shared")

        # ======== phase 1: M-rows = Wq[my 128 rows] @ WkT (fp16), AllGather
        with ExitStack() as ph1, nc.named_scope("p1_M"):
            wqp = ph1.enter_context(tc.tile_pool(name="wqp", bufs=1))
            ps1 = ph1.enter_context(tc.tile_pool(name="ps1", bufs=1, space="PSUM"))
            wqs = wqp.tile([128, FC, 128], F16, tag="wqs")  # 4KB/part
            wkT = wqp.tile([128, FC, C], F16, tag="wkT")  # 32KB/part
            # wkT split 3 ways by queue speed (~130/130/64 GB/s)
            nc.sync.dma_start(out=wqs[:, :, :], in_=wqs_ext[:, :, :])
            nc.scalar.dma_start(out=wkT[:, 6:12, :], in_=wkT_ext[:, 6:12, :])
            nc.sync.dma_start(out=wkT[:, :6, :], in_=wkT_ext[:, :6, :])
            nc.gpsimd.dma_start(out=wkT[:, 12:, :], in_=wkT_ext[:, 12:, :])

            # PE p-state warmup: dummy matmuls with no load dependencies
            wrmp = ph1.enter_context(tc.tile_pool(name="wrmp", bufs=1, space="PSUM"))
            wps = wrmp.tile([128, 128], F32, tag="wps")
            for _ in range(40):
                nc.tensor.matmul(wps[:, :], identf[:, :], identf[:, :])

            msl_sb = wqp.tile([128, C], F16, tag="msl_sb")  # 2KB/part
            mps = ps1.tile([128, C], F32, tag="mps")  # 2 banks
            forder = [6, 7, 0, 1, 8, 9, 2, 3, 10, 11, 4, 5, 12, 13, 14, 15]
            for i, f in enumerate(forder):
                for h in range(2):
                    nc.tensor.matmul(
                        mps[:, h * 512 : (h + 1) * 512],
                        wqs[:, f, :],
                        wkT[:, f, h * 512 : (h + 1) * 512],
                        start=(i == 0),
                        stop=(i == FC - 1),
                    )
            nc.vector.tensor_copy(out=msl_sb[:, :], in_=mps[:, :])
            nc.gpsimd.dma_start(out=msl_d[:, :], in_=msl_sb[:, :])
            # (the M AllGather itself is emitted in phase 2, on the gpsimd
            # queue after the wvT/wfs loads, so its wait doesn't stall them)

        # wvT halves ride the two fast HWDGE queues (SWDGE is ~64GB/s)

        # ======== phase 2: N2-slice = Wv @ Wf[:, my 256 cols], AllGather ===
        with ExitStack() as ph2, nc.named_scope("p2_N2"):
            wvp = ph2.enter_context(tc.tile_pool(name="wvp", bufs=1))
            ps2 = ph2.enter_context(tc.tile_pool(name="ps2", bufs=1, space="PSUM"))
            wvT = wvp.tile([128, FC, C], BF16, tag="wvT")  # 32KB/part
            wfs = wvp.tile([128, FC, 256], BF16, tag="wfs")  # 8KB/part
            nc.scalar.dma_start(out=wfs[:, :, :], in_=wfs_ext[:, :, :])
            nc.scalar.dma_start(out=wvT[:, 8:, :], in_=wvT_ext[:, 8:, :])
            nc.sync.dma_start(out=wvT[:, :8, :], in_=wvT_ext[:, :8, :])
            # sync: xT behind wvT-A (scores need it ~t=80)
            nc.sync.dma_start(out=xT[:, :, :], in_=xT_ext[:, :, :])
            # M AllGather on the gpsimd queue (its wait blocks nothing here)
            nc.gpsimd.collective_compute(
                "AllGather",
                mybir.AluOpType.bypass,
                replica_groups=ALL8,
                ins=[msl_d[:, :]],
                outs=[mall_d[:, :]],
            )
            n2s_sb = wvp.tile([128, CC, 256], F16, tag="n2s_sb")  # 4KB/part
            for ah in range(2):
                nps = ps2.tile([128, 4, 512], F32, tag="nps", name=f"nps{ah}")
                for f in range(FC):
                    for a4 in range(4):
                        nc.tensor.matmul(
                            nps[:, a4, :256],
                            wvT[:, f, (ah * 4 + a4) * 128 : (ah * 4 + a4 + 1) * 128],
                            wfs[:, f, :],
                            start=(f == 0),
                            stop=(f == FC - 1),
                        )
                for a4 in range(4):
                    nc.vector.tensor_copy(
                        out=n2s_sb[:, ah * 4 + a4, :], in_=nps[:, a4, :256]
                    )
            nc.scalar.dma_start(out=n2s_d[:, :], in_=n2s_sb[:, :, :])
            nc.gpsimd.collective_compute(
                "AllGather",
                mybir.AluOpType.bypass,
                replica_groups=ALL8,
                ins=[n2s_d[:, :]],
                outs=[n2all_d[:, :]],
            )

        # ======== phase 2b: xb[t-part, c] = PE-transpose of xT ==============
        # (replaces a 4MB HBM load; fills the PE while the M-AllGather runs)
        with ExitStack() as ph2b, nc.named_scope("p2b_xb"):
            psx = ph2b.enter_context(tc.tile_pool(name="psx", bufs=2, space="PSUM"))
            for s in range(NB):
                for cc in range(CC):
                    xtp = psx.tile(
                        [128, 128], F16, tag="xtp", name=f"xtp{s}_{cc}"
                    )
                    nc.tensor.transpose(
                        xtp[:, :], xT[:, cc, s * 128 : (s + 1) * 128], identf[:, :]
                    )
                    nc.vector.tensor_copy(
                        out=xb[:, s, cc * 128 : (cc + 1) * 128], in_=xtp[:, :]
                    )
            for i in range(48):  # reader-less fillers: keep the clock hot
                xf = psx.tile([128, 128], F16, tag="xtp", name=f"xfill{i}")
                nc.tensor.transpose(
                    xf[:, :], xT[:, i % 8, :128], identf[:, :]
                )

        # ======== phase 3: uT = (xq M).T  [c2-chunk, owned-t] fp16 =========
        # post pool reuses the weight pools' SBUF space (they are closed).
        post = root.enter_context(tc.tile_pool(name="post", bufs=1))
        mM = post.tile([128, CC, C], F16, tag="mM")  # 16KB/part
        n2 = post.tile([128, NCORES, F], F16, tag="n2")  # 32KB/part
        with ExitStack() as ph3, nc.named_scope("p3_uT"):
            ps3 = ph3.enter_context(tc.tile_pool(name="ps3", bufs=4, space="PSUM"))
            # mall_d row r = M row r -> mM[p, c1, :] = M[c1*128+p, :]
            for mh, eng in ((0, nc.scalar), (1, nc.sync)):  # one half per queue
                eng.dma_start(
                    out=mM[:, :, mh * 512 : (mh + 1) * 512],
                    in_=mall_d[:, mh * 512 : (mh + 1) * 512].rearrange(
                        "(c p) w -> p c w", p=128
                    ),
                )
            def xq_ap(c1, tt):
                """owned (odd) device blocks {1,3,5,7}+8*tt of xT[:, c1, :]"""
                base = xT[:, c1, :]
                return bass.AP(
                    tensor=base.tensor,
                    offset=base.offset + 128 + tt * 1024,
                    ap=[base.ap[0], [256, 4], [1, 128]],
                )

            for tt in (1, 0):  # tt=1 first: it feeds stages 0-3
                for c2 in range(CC):
                    ups = ps3.tile(
                        [128, 512], F32, tag="ups", name=f"ups{tt}_{c2}"
                    )
                    for c1 in range(CC):
                        nc.tensor.matmul(
                            ups[:, :],
                            mM[:, c1, c2 * 128 : (c2 + 1) * 128],
                            xq_ap(c1, tt),
                            start=(c1 == 0),
                            stop=(c1 == CC - 1),
                        )
                    nc.vector.tensor_copy(
                        out=uT[:, c2, tt * 512 : (tt + 1) * 512], in_=ups[:, :]
                    )

        # small loads (act queue is idle now); n2 gather cache on sync.
        # n2[p, gi, cj*256+w] = N2[cj*128+p, gi*256+w] -- row-block gi of
        # n2all_d lands contiguously (4KB runs per partition).
        nc.scalar.dma_start(out=m2[:, :, :], in_=m2_ext[:, :, :])
        bf_ap = bf_ext[:]
        nc.scalar.dma_start(
            out=bfb,
            in_=bass.AP(
                tensor=bf_ap.tensor,
                offset=bf_ap.offset,
                ap=[[0, 128]] + list(bf_ap.ap),
            ),
        )
        nc.sync.dma_start(
            out=n2[:, :, :],
            in_=n2all_d[:, :].rearrange("(g p) w -> p g w", p=128),
        )

        # ======== phase 5: attention over owned blocks =====================
        with ExitStack() as ph6:
            st6 = ph6.enter_context(tc.tile_pool(name="st6", bufs=2))
            small = ph6.enter_context(tc.tile_pool(name="small", bufs=4))
            ps_s = ph6.enter_context(tc.tile_pool(name="ps_s", bufs=2, space="PSUM"))
            ps_big = ph6.enter_context(tc.tile_pool(name="ps_big", bufs=2, space="PSUM"))
            ps_t = ph6.enter_context(tc.tile_pool(name="ps_t", bufs=2, space="PSUM"))

            def softmax_stage(k):
                with nc.named_scope(f"p5_sm{k}"):
                    return _softmax_stage(k)

            def _softmax_stage(k):
                """scores -> masked SBUF copy -> exp -> bf16 probs"""
                ek = E[k]
                scols = ek * 128
                s_sb = st6.tile([128, T], F32, tag="s_sb", name=f"s_sb{k}", bufs=2)
                m0 = scols - 256  # mask window start
                for h0 in range(0, scols, 512):
                    hw = min(512, scols - h0)
                    sps = ps_s.tile([128, 512], F32, tag="sps", name=f"sps{k}_{h0}")
                    for c2 in range(CC):
                        nc.tensor.matmul(
                            sps[:, :hw],
                            uT[:, c2, (7 - k) * 128 : (8 - k) * 128],
                            xT[:, c2, h0 : h0 + hw],
                            start=(c2 == 0),
                            stop=(c2 == CC - 1),
                        )
                    plain = min(hw, max(0, m0 - h0))
                    if plain > 0:
                        nc.vector.tensor_copy(
                            out=s_sb[:, h0 : h0 + plain], in_=sps[:, :plain]
                        )
                    if plain < hw:
                        nc.vector.tensor_add(
                            s_sb[:, h0 + plain : h0 + hw],
                            sps[:, plain:hw],
                            m2[:, k, h0 + plain - m0 : h0 + hw - m0],
                        )
                negmax = small.tile(
                    [128, 1], F32, tag="negmax", name=f"negmax{k}", bufs=6
                )
                nc.vector.tensor_reduce(
                    out=negmax,
                    in_=s_sb[:, :scols],
                    axis=mybir.AxisListType.X,
                    op=mybir.AluOpType.max,
                    negate=True,
                )
                psb = st6.tile([128, T], F16, tag="psb", name=f"psb{k}", bufs=2)
                rsum = small.tile([128, 1], F32, tag="rsum", name=f"rsum{k}", bufs=6)
                nc.scalar.activation(
                    out=psb[:, :scols],
                    in_=s_sb[:, :scols],
                    func=mybir.ActivationFunctionType.Exp,
                    bias=negmax,
                    scale=1.0,
                    accum_out=rsum,
                )
                rinv = small.tile([128, 1], F32, tag="rinv", name=f"rinv{k}", bufs=6)
                nc.vector.reciprocal(out=rinv, in_=rsum)
                return psb, rinv

            def transpose_stage(k, psb):
                with nc.named_scope(f"p5_tr{k}"):
                    for sc in range(E[k]):
                        pt = ps_t.tile([128, 128], F16, tag="pt", name=f"pt{k}_{sc}")
                        nc.tensor.transpose(
                            pt[:, :], psb[:, sc * 128 : (sc + 1) * 128], identf[:, :]
                        )
                        nc.scalar.activation(
                            out=ptall[:, EOFF[k] + sc, :],
                            in_=pt[:, :],
                            func=mybir.ActivationFunctionType.Copy,
                        )

            def o1_stage(k):
                """o1 = P @ x [q,1024] -> bf16 -> PE transpose -> o1T [c,q]"""
                with nc.named_scope(f"p5_o1_{k}"):
                    ek = E[k]
                    o1ps = ps_big.tile([128, 1024], F32, tag="big", name=f"o1ps{k}")
                    for sc in range(ek):
                        for h in range(2):  # same ptall stationary twice
                            nc.tensor.matmul(
                                o1ps[:, h * 512 : (h + 1) * 512],
                                ptall[:, EOFF[k] + sc, :],
                                xb[:, sc, h * 512 : (h + 1) * 512],
                                start=(sc == 0),
                                stop=(sc == ek - 1),
                            )
                    o1sb = st6.tile([128, C], F16, tag="o1sb", name=f"o1sb{k}", bufs=1)
                    nc.scalar.activation(
                        out=o1sb[:, :],
                        in_=o1ps[:, :],
                        func=mybir.ActivationFunctionType.Copy,
                    )
                    o1T = st6.tile(
                        [128, CC, 128], F16, tag="o1T", name=f"o1T{k}", bufs=1
                    )
                    for cj in range(CC):
                        o1tp = ps_t.tile(
                            [128, 128], F16, tag="pt", name=f"o1tp{k}_{cj}"
                        )
                        nc.tensor.transpose(
                            o1tp[:, :], o1sb[:, cj * 128 : (cj + 1) * 128], identf[:, :]
                        )
                        nc.scalar.activation(
                            out=o1T[:, cj, :],
                            in_=o1tp[:, :],
                            func=mybir.ActivationFunctionType.Copy,
                        )
                    return o1T

            def out_stage(k, o1T, rinv):
                """out = (o1T @ N2) * rinv + bf, store bf16"""
                with nc.named_scope(f"p5_out{k}"):
                    orow = st6.tile([128, F], BF16, tag="orow", name=f"orow{k}", bufs=1)
                    for h2 in range(2):
                        ops = ps_big.tile(
                            [128, 1024], F32, tag="big", name=f"ops{k}_{h2}"
                        )
                        for cj in range(CC):
                            for g2 in range(2):  # same o1T stationary twice
                                gi = (h2 * 2 + g2) * 2
                                nc.tensor.matmul(
                                    ops[:, g2 * 512 : (g2 + 1) * 512],
                                    o1T[:, cj, :],
                                    n2[:, gi : gi + 2, cj * 256 : (cj + 1) * 256],
                                    start=(cj == 0),
                                    stop=(cj == CC - 1),
                                )
                        nc.vector.scalar_tensor_tensor(
                            out=orow[:, h2 * 1024 : (h2 + 1) * 1024],
                            in0=ops,
                            scalar=rinv,
                            in1=bfb[:, h2 * 1024 : (h2 + 1) * 1024],
                            op0=mybir.AluOpType.mult,
                            op1=mybir.AluOpType.add,
                        )
                    nc.sync.dma_start(out=out_ext[k], in_=orow)

            # pipeline: S_{k+2} runs on PE between stage-k transposes and
            # stage-(k+1) work so exp latency never stalls the PE.
            sm = {}
            sm[0] = softmax_stage(0)
            sm[1] = softmax_stage(1)
            for k in range(8):
                psb, rinv = sm.pop(k)
                transpose_stage(k, psb)
                o1T = o1_stage(k)
                out_stage(k, o1T, rinv)
                if k + 2 < 8:
                    sm[k + 2] = softmax_stage(k + 2)

    nc.finalize()
    return nc


def _get_program():
    if "nc" not in _CACHE:
        _CACHE["nc"] = _build_program()
    return _CACHE["nc"]


def _sb(a, p=128):
    """[n*p, w] -> SBUF layout [p, n, w] (partition-major)."""
    n = a.shape[0] // p
    return np.ascontiguousarray(a.reshape(n, p, a.shape[1]).transpose(1, 0, 2))


def _make_in_maps(x, Wq, Wk, Wv, Wf, bf):
    x = np.ascontiguousarray(x, dtype=np.float32)
    WqT = np.ascontiguousarray(np.asarray(Wq, dtype=np.float32).T)
    WkT16 = np.ascontiguousarray(np.asarray(Wk, dtype=np.float32).T).astype(np.float16)
    WvTb = np.ascontiguousarray(np.asarray(Wv, dtype=np.float32).T).astype(
        ml_dtypes.bfloat16
    )
    Wfb = np.asarray(Wf, dtype=np.float32).astype(ml_dtypes.bfloat16)
    bf = np.ascontiguousarray(bf, dtype=np.float32)
    wkT_sb = _sb(WkT16)
    wvT_sb = _sb(WvTb)
    in_maps = []
    for core in range(NCORES):
        b, h = core // 2, core % 2
        xb = x[b]
        if h == 1:  # pair-swap adjacent 128-row blocks: device block 2j+1
            xb = np.ascontiguousarray(  # holds true even block 2j
                xb.reshape(8, 2, 128, C)[:, ::-1].reshape(T, C)
            )
        mask2 = np.zeros((8, 128, 256), dtype=np.float32)  # cast below
        for k in range(8):
            tq = 15 - 2 * k - h  # true query block of stage k
            sd = (E[k] - 2) * 128 + np.arange(256)[None, :]  # device key col
            s = (np.right_shift(sd, 7) ^ h) * 128 + (sd & 127)  # true key
            t = tq * 128 + np.arange(128)[:, None]
            mask2[k] = np.where(s <= t, 0.0, NEG).astype(np.float32)
        in_maps.append(
            {
                "xTin": _sb(np.ascontiguousarray(xb.T).astype(np.float16)),
                "mask2": np.ascontiguousarray(
                    mask2.astype(ml_dtypes.bfloat16).transpose(1, 0, 2)
                ),
                "WqTs": _sb(
                    np.ascontiguousarray(
                        WqT[:, core * 128 : (core + 1) * 128]
                    ).astype(np.float16)
                ),
                "WkTf": wkT_sb,
                "WvTb": wvT_sb,
                "Wfs": _sb(
                    np.ascontiguousarray(Wfb[:, core * 256 : (core + 1) * 256])
                ),
                "bf": bf,
            }
        )
    return in_maps


def run_on_hw(inputs, trace=False, trace_cores=None):
    nc = _get_program()
    in_maps = _make_in_maps(**inputs)
    res = run_bass_kernel_spmd(
        nc, in_maps, list(range(NCORES)), trace=trace, trace_cores=trace_cores
    )
    out = np.empty((B, T, F), dtype=np.float32)
    for core in range(NCORES):
        b, h = core // 2, core % 2
        o = res.results[core]["out"]  # [8, 128, F] bf16
        for k in range(8):
            blk = 15 - 2 * k - h  # true query block of stage k
            out[b, blk * 128 : (blk + 1) * 128, :] = o[k].astype(np.float32)
    return out, res


def kernel(x, Wq, Wk, Wv, Wf, bf):
    out, _ = run_on_hw(dict(x=x, Wq=Wq, Wk=Wk, Wv=Wv, Wf=Wf, bf=bf))
    return out
